# revision 17
# baseline (speedup 1.0000x reference)
"""Trainium2 Bass kernel for nn_MemoryLayerAttention_27917287424099.

Mathematical collapse of the reference RNN:
  - The conductance-ODE "pot" state receives zero external input
    (neuron_inputs = zeros), starts at the same (0, 1) pair in every one
    of the BQ*MC cells, and its update depends only on itself and
    hardcoded constants.  It therefore evolves identically in every cell
    and is a compile-time-constant scalar trajectory.
  - Only the LAST scan step's LSTM output is returned (ys[-1]), and steps
    interact only through pot, so steps 0..6's attention/LSTM outputs are
    dead code.
  - Hence the whole model == one attention + LSTM-gate step evaluated on
    x_7 = concat(queries[b,q], values[b,7]) with the memory matrix equal
    to the constant p0 (pot[...,0] after 7*2 Euler iterations) broadcast
    everywhere.
  - Of the LSTM gate pre-activation z (4*1184 cols), only zi/zg/zo's
    first 1024 columns are used (zf and the tail are dead).

Sharding: batch (128) lives on the SBUF partition dim; the replicated
attention preamble is computed on every core, and the 1024 output
columns of the LSTM matmul + gate math are sharded 128/core across the
8 cores (each core gets its own 3*128-column slice of Wx/bl).

Perf notes baked in:
  - fp32 matmuls run as LOW_HIGH double passes on trn2; all TensorE
    operands are bf16 here (single pass), PSUM accumulation stays fp32.
    Measured end-to-end error vs the f32 reference: ~5e-3.
  - each independent matmul accumulation group owns its own PSUM tile
    (two groups sharing a PSUM bank crash the device).
  - inputs arrive in 5 packed DMAs (DMA issue is serialized on SyncE at
    ~0.7us apiece, so count matters, not bytes).
  - sigmoid(x) = 0.5*(1+tanh(x/2)) keeps every ACT function in the
    exp_and_others table set: one ACT_TABLE_LOAD instead of two.
"""

import os
import numpy as np
import ml_dtypes

BF16 = ml_dtypes.bfloat16

DIM = 16
EMB = 64
ROWS = 64
RH = 2
OUT = 1024
UNITS = 1184
B, Q, V = 8, 16, 8
BQ = B * Q
DSTEPS = 2
N_CORES = 8
CPC = OUT // N_CORES  # columns per core = 128
SCALE = float(1.0 / np.sqrt(np.float32(EMB)))

# ---------------------------------------------------------------------------
# compile-time constants (derived only from constants hardcoded in the model)
# ---------------------------------------------------------------------------


def _pot_scalar():
    """p0 = pot[..., 0] as read by scan step 7 (after 14 f32 Euler steps)."""
    cond = np.array([0.07915332, 1.0334609, 1.3365093, 0.4505964], np.float32)
    mean = np.array([0.5, 0.07879465, 0.06618887, 0.0], np.float32)
    std = np.array([100.0, 100.0, 100.0, 1.0], np.float32)
    tgt = np.array([1.5931877, 1.4378392, 0.0, 0.0], np.float32)
    part = np.float32(1.5573331 / DSTEPS)

    def sig(x):
        return np.float32(1.0) / (np.float32(1.0) + np.exp(-x, dtype=np.float32))

    p = np.array([0.0, 1.0], np.float32)
    inp = np.zeros(2, np.float32)
    for _ in range((V - 1) * DSTEPS):
        pre = np.stack([inp, p, p[::-1], np.full_like(p, np.inf)], -1)
        s = sig(std * (pre - mean))
        curr = cond * s * (tgt - p[:, None])
        p = (p + curr.sum(-1, dtype=np.float32) * part).astype(np.float32)
    return float(p[0])


P0 = _pot_scalar()


def _pe_table():
    L = ROWS + 1
    pos = np.arange(L, dtype=np.float32)[:, None]
    i = np.arange(EMB)[None, :]
    ang = pos / np.power(10000.0, (2 * (i // 2)) / EMB)
    return np.where(i % 2 == 0, np.sin(ang), np.cos(ang)).astype(np.float32)


PE = _pe_table()  # (65, 64)

# packed-input column offsets
# pk33 (33, 192): x7aT | WiA
# pk65 (65, 768): WqA | WkA | WvA | WxA(384)
# pk64 (64, 192): PET1 | WoP_h0 | WoP_h1
# pk128 (128, 259): Wm_chunk0 | Wm_chunk1 | ident | ones | hmask(2)
# pkb  (64, 2) f32: bm | bo

_CACHE = {}
LAST_EXEC_TIME_NS = None


def _build():
    import concourse.bacc as bacc
    import concourse.tile as tile
    from concourse import mybir

    F32 = mybir.dt.float32
    BF = mybir.dt.bfloat16
    AF = mybir.ActivationFunctionType
    ALU = mybir.AluOpType
    AX = mybir.AxisListType

    nc = bacc.Bacc(None, target_bir_lowering=False, debug=False)

    d_pk33 = nc.declare_dram_parameter("pk33", [33, 192], BF, isOutput=False)
    d_pk65 = nc.declare_dram_parameter("pk65", [EMB + 1, 768], BF, isOutput=False)
    d_pk64 = nc.declare_dram_parameter("pk64", [EMB, 192], BF, isOutput=False)
    d_pk128 = nc.declare_dram_parameter("pk128", [128, 259], BF, isOutput=False)
    d_pkb = nc.declare_dram_parameter("pkb", [EMB, 2], F32, isOutput=False)
    d_out = nc.declare_dram_parameter("out", [BQ, CPC], F32, isOutput=True)

    with tile.TileContext(nc) as tc:
        with (
            tc.tile_pool(name="sb", bufs=1) as sb,
            tc.tile_pool(name="ps", bufs=1, space="PSUM") as ps,
        ):
            # ---- packed loads, ordered by first use --------------------
            pk33 = sb.tile([33, 192], BF, tag="pk33", name="pk33")
            nc.sync.dma_start(out=pk33[:], in_=d_pk33[:])
            pk65 = sb.tile([EMB + 1, 768], BF, tag="pk65", name="pk65")
            nc.scalar.dma_start(out=pk65[:], in_=d_pk65[:])
            pk128 = sb.tile([128, 259], BF, tag="pk128", name="pk128")
            nc.sync.dma_start(out=pk128[:], in_=d_pk128[:])
            pkb = sb.tile([EMB, 2], F32, tag="pkb", name="pkb")
            nc.sync.dma_start(out=pkb[:], in_=d_pkb[:])
            pk64 = sb.tile([EMB, 192], BF, tag="pk64", name="pk64")
            nc.gpsimd.dma_start(out=pk64[:], in_=d_pk64[:])

            x7aT = pk33[:, 0:128]
            WiA = pk33[:, 128:192]
            WqA = pk65[:, 0:128]
            WkA = pk65[:, 128:256]
            WvA = pk65[:, 256:384]
            WxA = pk65[:, 384:768]
            PET1 = pk64[:, 0:64]
            WoP = [pk64[:, 64 + h * EMB : 64 + (h + 1) * EMB] for h in range(RH)]
            WmC = [pk128[:, h * EMB : (h + 1) * EMB] for h in range(2)]
            ident = pk128[:, 128:256]
            ones = pk128[:, 256:257]
            hmask = pk128[:, 257:259]
            bm = pkb[:, 0:1]
            bo = pkb[:, 1:2]

            # warm the ACT table set early (Exp/Tanh load overlaps the DMAs)
            warm = sb.tile([128, 1], F32, tag="warm", name="warm")
            nc.vector.memset(warm[:], 0.0)
            warm2 = sb.tile([128, 1], F32, tag="warm2", name="warm2")
            nc.scalar.activation(warm2[:], warm[:], AF.Exp)

            # ---- aug0T = (x7 @ Wi + bi + PE0)^T, augmented with ones row
            emb_ps = ps.tile([EMB, BQ], F32, tag="mm", bufs=5, name="emb_ps")
            nc.tensor.matmul(emb_ps[:], lhsT=WiA, rhs=x7aT, start=True, stop=True)
            aug0T = sb.tile([EMB + 1, BQ], BF, tag="aug0T", name="aug0T")
            nc.scalar.copy(aug0T[0:EMB, :], emb_ps[:])
            nc.vector.memset(aug0T[EMB : EMB + 1, :], 1.0)

            # ---- m_vec = p0 * colsum(Wm) + bm  (per-partition, EMB rows)
            colsum_ps = ps.tile([EMB, 1], F32, tag="mm", bufs=5, name="colsum_ps")
            nc.tensor.matmul(
                colsum_ps[:], lhsT=WmC[0], rhs=ones, start=True, stop=False
            )
            nc.tensor.matmul(
                colsum_ps[:], lhsT=WmC[1], rhs=ones, start=False, stop=True
            )
            m_vec = sb.tile([EMB, 1], F32, tag="m_vec", name="m_vec")
            nc.scalar.activation(
                m_vec[:], colsum_ps[:], AF.Identity, bias=bm, scale=P0
            )

            # ---- augR = (m_vec + PE[1:].T), augmented with ones row -----
            augR = sb.tile([EMB + 1, ROWS], BF, tag="augR", name="augR")
            nc.vector.tensor_scalar_add(augR[0:EMB, :], PET1, m_vec[:])
            nc.vector.memset(augR[EMB : EMB + 1, :], 1.0)

            # ---- q / k0 / v0 -------------------------------------------
            q_ps = ps.tile([128, BQ], F32, tag="mm", bufs=5, name="q_ps")
            nc.tensor.matmul(q_ps[:], lhsT=WqA, rhs=aug0T[:], start=True, stop=True)
            qT = sb.tile([128, BQ], BF, tag="qT", name="qT")
            nc.scalar.mul(qT[:], q_ps[:], SCALE)  # fold attention scale into q

            k0_ps = ps.tile([128, BQ], F32, tag="mm", bufs=5, name="k0_ps")
            nc.tensor.matmul(k0_ps[:], lhsT=WkA, rhs=aug0T[:], start=True, stop=True)
            k0T = sb.tile([128, BQ], BF, tag="k0T", name="k0T")
            nc.vector.tensor_copy(k0T[:], k0_ps[:])

            # v0 batch-major: (128b, 128hk)
            v0_ps = ps.tile([BQ, 128], F32, tag="mm", bufs=5, name="v0_ps")
            nc.tensor.matmul(v0_ps[:], lhsT=aug0T[:], rhs=WvA, start=True, stop=True)
            v0bm = sb.tile([BQ, 128], BF, tag="v0bm", name="v0bm")
            nc.vector.tensor_copy(v0bm[:], v0_ps[:])

            # ---- K^T (k-major) and V (l-major) for the 64 memory rows ---
            kt_ps = ps.tile([128, ROWS], F32, tag="mm", bufs=5, name="kt_ps")
            nc.tensor.matmul(kt_ps[:], lhsT=WkA, rhs=augR[:], start=True, stop=True)
            ktT = sb.tile([128, ROWS], BF, tag="ktT", name="ktT")
            nc.vector.tensor_copy(ktT[:], kt_ps[:])

            vl_ps = ps.tile([ROWS, 128], F32, tag="mm", bufs=5, name="vl_ps")
            nc.tensor.matmul(vl_ps[:], lhsT=augR[:], rhs=WvA, start=True, stop=True)
            vl = sb.tile([ROWS, 128], BF, tag="vl", name="vl")
            nc.vector.tensor_copy(vl[:], vl_ps[:])

            # ---- attention logits --------------------------------------
            logR_ps = []
            for h in range(RH):
                lp = ps.tile([BQ, ROWS], F32, tag="mm", bufs=5, name=f"logR{h}")
                nc.tensor.matmul(
                    lp[:],
                    lhsT=qT[h * EMB : (h + 1) * EMB, :],
                    rhs=ktT[h * EMB : (h + 1) * EMB, :],
                    start=True,
                    stop=True,
                )
                logR_ps.append(lp)
            prod = sb.tile([128, BQ], BF, tag="prod", name="prod")
            nc.vector.tensor_mul(prod[:], qT[:], k0T[:])
            log0_ps = ps.tile([BQ, RH], F32, tag="mm", bufs=5, name="log0_ps")
            nc.tensor.matmul(log0_ps[:], lhsT=prod[:], rhs=hmask, start=True, stop=True)

            # ---- softmax over 65 positions per (b, h) -------------------
            # |logit| <= ~2 here, so no max-subtraction needed before exp
            e = sb.tile([BQ, RH, ROWS + 1], F32, tag="e", name="e")
            s0 = sb.tile([BQ, RH], F32, tag="s0", name="s0")
            sR = sb.tile([BQ, RH], F32, tag="sR", name="sR")
            for h in range(RH):
                nc.scalar.activation(
                    e[:, h, 0:1], log0_ps[:, h : h + 1], AF.Exp,
                    accum_out=s0[:, h : h + 1],
                )
                nc.scalar.activation(
                    e[:, h, 1:], logR_ps[h][:], AF.Exp, accum_out=sR[:, h : h + 1]
                )
            ssum = sb.tile([BQ, RH], F32, tag="ssum", name="ssum")
            nc.vector.tensor_add(ssum[:], s0[:], sR[:])
            rsum = sb.tile([BQ, RH], F32, tag="rsum", name="rsum")
            nc.vector.reciprocal(rsum[:], ssum[:])
            attn = sb.tile([BQ, RH, ROWS + 1], BF, tag="attn", name="attn")
            for h in range(RH):
                nc.vector.tensor_scalar_mul(
                    attn[:, h, :], e[:, h, :], rsum[:, h : h + 1]
                )

            # ---- ctx^T, laid out (64 k, 2 h, 128 b); all matmul operands
            # at base partition 0, one PSUM tile per matmul group ---------
            atT_sb = []
            for h in range(RH):
                atT_ps = ps.tile([ROWS, BQ], BF, tag="mm", bufs=5, name=f"atT{h}")
                nc.tensor.transpose(atT_ps[:], attn[:, h, 1:], ident)
                t = sb.tile([ROWS, BQ], BF, tag=f"atTs{h}", name=f"atTs{h}")
                nc.scalar.copy(t[:], atT_ps[:])
                atT_sb.append(t)
            ctxR_ps = []
            for h in range(RH):
                cp = ps.tile([EMB, BQ], F32, tag="ctx", bufs=2, name=f"ctxR{h}")
                nc.tensor.matmul(
                    cp[:],
                    lhsT=vl[:, h * EMB : (h + 1) * EMB],
                    rhs=atT_sb[h][:],
                    start=True,
                    stop=True,
                )
                ctxR_ps.append(cp)
            # l=0 term: attn0 * v0 batch-major, one full transpose
            ctx0bm = sb.tile([BQ, 128], BF, tag="ctx0bm", name="ctx0bm")
            for h in range(RH):
                nc.vector.tensor_scalar(
                    ctx0bm[:, h * EMB : (h + 1) * EMB],
                    v0bm[:, h * EMB : (h + 1) * EMB],
                    e[:, h, 0:1],
                    rsum[:, h : h + 1],
                    op0=ALU.mult,
                    op1=ALU.mult,
                )
            c0p = ps.tile([128, BQ], BF, tag="mm", bufs=5, name="ctx0T_ps")
            nc.tensor.transpose(c0p[:], ctx0bm[:], ident)
            ctx0T_sb = sb.tile([128, BQ], F32, tag="ctx0T_sb", name="ctx0T_sb")
            nc.scalar.copy(ctx0T_sb[:], c0p[:])
            ctx = sb.tile([EMB, RH, BQ], BF, tag="ctx_sb", name="ctx")
            for h in range(RH):
                nc.vector.tensor_add(
                    ctx[:, h, :],
                    ctxR_ps[h][:],
                    ctx0T_sb[h * EMB : (h + 1) * EMB, :],
                )

            # ---- o^T = sum_h Wo[h]^T ctx[h] + bo, augmented ones row ----
            oT_ps = ps.tile([EMB, BQ], F32, tag="mm", bufs=5, name="oT_ps")
            for h in range(RH):
                nc.tensor.matmul(
                    oT_ps[:],
                    lhsT=WoP[h],
                    rhs=ctx[:, h, :],
                    start=(h == 0),
                    stop=(h == RH - 1),
                )
            oTa = sb.tile([EMB + 1, BQ], BF, tag="oTa", name="oTa")
            nc.scalar.activation(oTa[0:EMB, :], oT_ps[:], AF.Identity, bias=bo)
            nc.vector.memset(oTa[EMB : EMB + 1, :], 1.0)

            # ---- z = o @ WxA + bl  (this core's 3*128 columns) ----------
            z_ps = ps.tile([BQ, 3 * CPC], F32, tag="z", bufs=1, name="z_ps")
            nc.tensor.matmul(z_ps[:], lhsT=oTa[:], rhs=WxA, start=True, stop=True)

            # ---- gates via tanh only (one ACT table set):
            # sig(x) = 0.5*(1+tanh(x/2))
            # out = sig(zo)*tanh(sig(zi)*tanh(zg))
            #     = 0.5*(t_o+1)*tanh(0.5*(t_i+1)*t_g)
            t_i = sb.tile([BQ, CPC], F32, tag="t_i", name="t_i")
            nc.scalar.activation(t_i[:], z_ps[:, 0:CPC], AF.Tanh, scale=0.5)
            t_g = sb.tile([BQ, CPC], F32, tag="t_g", name="t_g")
            nc.scalar.activation(t_g[:], z_ps[:, CPC : 2 * CPC], AF.Tanh)
            t_o = sb.tile([BQ, CPC], F32, tag="t_o", name="t_o")
            nc.scalar.activation(t_o[:], z_ps[:, 2 * CPC : 3 * CPC], AF.Tanh, scale=0.5)
            c2 = sb.tile([BQ, CPC], F32, tag="c2", name="c2")
            nc.vector.scalar_tensor_tensor(
                c2[:], t_i[:], 1.0, t_g[:], op0=ALU.add, op1=ALU.mult
            )
            tanh_c = sb.tile([BQ, CPC], F32, tag="tanh_c", name="tanh_c")
            nc.scalar.activation(tanh_c[:], c2[:], AF.Tanh, scale=0.5)
            out2 = sb.tile([BQ, CPC], F32, tag="out2", name="out2")
            nc.vector.scalar_tensor_tensor(
                out2[:], t_o[:], 1.0, tanh_c[:], op0=ALU.add, op1=ALU.mult
            )
            out_sb = sb.tile([BQ, CPC], F32, tag="out_sb", name="out_sb")
            nc.vector.tensor_scalar_mul(out_sb[:], out2[:], 0.5)

            nc.sync.dma_start(out=d_out[:], in_=out_sb[:])

    nc.compile()
    return nc




def _build_raw():
    """Hand-scheduled raw-bass version: 5 engine streams, explicit
    semaphores, manual PSUM bank reuse.  Dependency chain identical to
    the Tile build; ~3x fewer instructions (no vector-clock sem storm,
    no double all-engine exit barrier)."""
    import concourse.bacc as bacc
    import concourse.bass as bass
    from concourse import mybir
    from contextlib import ExitStack

    F32 = mybir.dt.float32
    BF = mybir.dt.bfloat16
    AF = mybir.ActivationFunctionType
    ALU = mybir.AluOpType

    nc = bacc.Bacc(None, target_bir_lowering=False, debug=False)

    d_pk33 = nc.declare_dram_parameter("pk33", [33, 192], BF, isOutput=False)
    d_pk65 = nc.declare_dram_parameter("pk65", [EMB + 1, 768], BF, isOutput=False)
    d_pk64 = nc.declare_dram_parameter("pk64", [EMB, 192], BF, isOutput=False)
    d_pk128 = nc.declare_dram_parameter("pk128", [128, 259], BF, isOutput=False)
    d_pkb = nc.declare_dram_parameter("pkb", [EMB, 2], F32, isOutput=False)
    d_out = nc.declare_dram_parameter("out", [BQ, CPC], F32, isOutput=True)

    ctx_mgr = ExitStack()
    sb = lambda shape, dt, name: ctx_mgr.enter_context(
        nc.sbuf_tensor("s_" + name, shape, dt)
    )
    psum = lambda name: ctx_mgr.enter_context(
        nc.psum_tensor(name, [128, 512], F32)
    )
    sem = lambda name: ctx_mgr.enter_context(nc.semaphore(name))

    with ctx_mgr:
        pk33 = sb([33, 192], BF, "pk33")
        pk65 = sb([EMB + 1, 768], BF, "pk65")
        pk64 = sb([EMB, 192], BF, "pk64")
        pk128 = sb([128, 259], BF, "pk128")
        pkb = sb([EMB, 2], F32, "pkb")
        warm = sb([128, 1], F32, "warm")
        warm2 = sb([128, 1], F32, "warm2")
        aug0T = sb([EMB + 1, BQ], BF, "aug0T")
        m_vec = sb([EMB, 1], F32, "m_vec")
        augR = sb([EMB + 1, ROWS], BF, "augR")
        qT = sb([128, BQ], BF, "qT")
        k0T = sb([128, BQ], BF, "k0T")
        v0bm = sb([BQ, 128], BF, "v0bm")
        ktT = sb([128, ROWS], BF, "ktT")
        vl = sb([ROWS, 128], BF, "vl")
        prod = sb([128, BQ], BF, "prod")
        e = sb([BQ, RH, ROWS + 1], F32, "e")
        sR = sb([BQ, RH], F32, "sR")
        ssum = sb([BQ, RH], F32, "ssum")
        rsum = sb([BQ, RH], F32, "rsum")
        attn = sb([BQ, RH, ROWS + 1], BF, "attn")
        atTs = [sb([ROWS, BQ], BF, f"atTs{h}") for h in range(RH)]
        ctx0T_sb = sb([128, BQ], BF, "ctx0T_sb")
        ctxT = sb([EMB, RH, BQ], BF, "ctxT")
        oTa = sb([EMB + 1, BQ], BF, "oTa")
        t_i = sb([BQ, CPC], F32, "t_i")
        t_g = sb([BQ, CPC], F32, "t_g")
        t_o = sb([BQ, CPC], F32, "t_o")
        c2 = sb([BQ, CPC], F32, "c2")
        tanh_c = sb([BQ, CPC], F32, "tanh_c")
        out2 = sb([BQ, CPC], F32, "out2")
        out_sb = sb([BQ, CPC], F32, "out_sb")
        ctx0bm_full = sb([BQ, 128], BF, "ctx0bm")

        pb = [psum(f"pb{i}") for i in range(8)]

        def bfv(bank):  # bf16 view of a psum bank
            return bank[:].bitcast(BF)

        # psum views (bank -> tenants, reuse proven safe by sem order)
        emb_ps = pb[0][0:EMB, 0:BQ]
        logR0_ps = pb[0][:, 0:ROWS]
        colsum_ps = pb[1][0:EMB, 0:1]
        logR1_ps = pb[1][:, 0:ROWS]
        q_ps = pb[2][:, 0:BQ]
        log0_ps = pb[2][:, 0:RH]
        ctx0T_ps = bfv(pb[2])[:, 0:BQ]
        k0_ps = pb[3][:, 0:BQ]
        atT0_ps = bfv(pb[3])[0:ROWS, 0:BQ]
        v0_ps = pb[4][:, 0:128]
        atT1_ps = bfv(pb[4])[0:ROWS, 0:BQ]
        kt_ps = pb[5][:, 0:ROWS]
        ctxR0_ps = pb[5][0:EMB, 0:BQ]
        vl_ps = pb[6][0:ROWS, 0:128]
        ctxR1_ps = pb[6][0:EMB, 0:BQ]
        oT_ps = pb[7][0:EMB, 0:BQ]
        z_ps = pb[7][:, 0 : 3 * CPC]

        d33, d128, dkb, d65, d64, dout = (
            sem("d33"), sem("d128"), sem("dkb"), sem("d65"), sem("d64"), sem("dout")
        )
        ts, asem, vs = sem("ts"), sem("asem"), sem("vs")

        WqA = pk65[:, 0:128]
        WkA = pk65[:, 128:256]
        WvA = pk65[:, 256:384]
        WxA = pk65[:, 384:768]
        x7aT = pk33[:, 0:128]
        WiA = pk33[:, 128:192]
        PET1 = pk64[:, 0:64]
        WoP = [pk64[:, 64 + h * EMB : 64 + (h + 1) * EMB] for h in range(RH)]
        WmC = [pk128[:, h * EMB : (h + 1) * EMB] for h in range(2)]
        ident = pk128[:, 128:256]
        ones = pk128[:, 256:257]
        hmask = pk128[:, 257:259]
        bmv = pkb[:, 0:1]
        bov = pkb[:, 1:2]

        with nc.Block() as block:

            @block.sync
            def _(s):
                s.dma_start(out=pk33[:], in_=d_pk33[:]).then_inc(d33, 16)
                s.dma_start(out=pk128[:], in_=d_pk128[:]).then_inc(d128, 16)
                s.dma_start(out=pkb[:], in_=d_pkb[:]).then_inc(dkb, 16)
                s.wait_ge(vs, 18)
                s.dma_start(out=d_out[:], in_=out_sb[:]).then_inc(dout, 16)
                s.wait_ge(dout, 16)

            @block.gpsimd
            def _(g):
                g.dma_start(out=pk64[:], in_=d_pk64[:]).then_inc(d64, 16)

            @block.scalar
            def _(a):
                a.dma_start(out=pk65[:], in_=d_pk65[:]).then_inc(d65, 16)
                a.memzero(warm[:]).then_inc(asem, 1)  # asem 1 (zero-bias tile)
                a.wait_ge(asem, 1)
                a.activation(warm2[:], warm[:], AF.Exp, bias=warm[:])  # table warm
                zb = warm[:]
                # aug0T rows 0..63 (row 64 ones comes from DVE)
                a.wait_ge(ts, 1)
                a.copy(aug0T[0:EMB, :], emb_ps).then_inc(asem, 1)  # asem 2
                a.wait_ge(ts, 2)
                a.wait_ge(dkb, 16)
                a.activation(
                    m_vec[:], colsum_ps, AF.Identity, bias=bmv, scale=P0
                ).then_inc(asem, 1)  # asem 3
                a.wait_ge(ts, 3)
                a.mul(qT[:], q_ps, SCALE).then_inc(asem, 1)  # asem 4
                a.wait_ge(ts, 8)
                a.activation(
                    e[:, 0, 1:], logR0_ps, AF.Exp, bias=zb, accum_out=sR[:, 0:1]
                ).then_inc(asem, 1)  # asem 5
                a.wait_ge(ts, 9)
                a.activation(
                    e[:, 1, 1:], logR1_ps, AF.Exp, bias=zb, accum_out=sR[:, 1:2]
                ).then_inc(asem, 1)  # asem 6
                a.wait_ge(ts, 10)
                a.activation(e[:, :, 0], log0_ps, AF.Exp, bias=zb).then_inc(
                    asem, 1
                )  # asem 7
                a.wait_ge(ts, 11)
                a.copy(atTs[0][:], atT0_ps).then_inc(asem, 1)  # asem 8
                a.wait_ge(ts, 12)
                a.copy(atTs[1][:], atT1_ps).then_inc(asem, 1)  # asem 9
                a.wait_ge(ts, 15)
                a.copy(ctx0T_sb[:], ctx0T_ps).then_inc(asem, 1)  # asem 10
                a.wait_ge(ts, 16)
                a.activation(
                    oTa[0:EMB, :], oT_ps, AF.Identity, bias=bov
                ).then_inc(asem, 1)  # asem 11
                a.wait_ge(ts, 17)
                a.activation(t_i[:], z_ps[:, 0:CPC], AF.Tanh, bias=zb, scale=0.5).then_inc(
                    asem, 1
                )  # asem 12
                a.activation(
                    t_g[:], z_ps[:, CPC : 2 * CPC], AF.Tanh, bias=zb
                ).then_inc(asem, 1)  # asem 13
                a.activation(
                    t_o[:], z_ps[:, 2 * CPC : 3 * CPC], AF.Tanh, bias=zb, scale=0.5
                ).then_inc(asem, 1)  # asem 14
                a.wait_ge(vs, 16)
                a.activation(tanh_c[:], c2[:], AF.Tanh, bias=zb, scale=0.5).then_inc(
                    asem, 1
                )  # asem 15

            @block.tensor
            def _(t):
                t.wait_ge(d33, 16)
                t.matmul(emb_ps, lhsT=WiA, rhs=x7aT, start=True, stop=True).then_inc(
                    ts, 1
                )  # ts 1
                t.wait_ge(d128, 16)
                t.matmul(colsum_ps, lhsT=WmC[0], rhs=ones, start=True, stop=False)
                t.matmul(
                    colsum_ps, lhsT=WmC[1], rhs=ones, start=False, stop=True
                ).then_inc(ts, 1)  # ts 2
                t.wait_ge(asem, 2)
                t.wait_ge(vs, 1)
                t.wait_ge(d65, 16)
                t.matmul(q_ps, lhsT=WqA, rhs=aug0T[:], start=True, stop=True).then_inc(
                    ts, 1
                )  # ts 3
                t.matmul(k0_ps, lhsT=WkA, rhs=aug0T[:], start=True, stop=True).then_inc(
                    ts, 1
                )  # ts 4
                t.matmul(v0_ps, lhsT=aug0T[:], rhs=WvA, start=True, stop=True).then_inc(
                    ts, 1
                )  # ts 5
                t.wait_ge(vs, 2)
                t.matmul(kt_ps, lhsT=WkA, rhs=augR[:], start=True, stop=True).then_inc(
                    ts, 1
                )  # ts 6
                t.matmul(vl_ps, lhsT=augR[:], rhs=WvA, start=True, stop=True).then_inc(
                    ts, 1
                )  # ts 7
                t.wait_ge(asem, 4)
                t.wait_ge(vs, 5)
                t.matmul(
                    logR0_ps,
                    lhsT=qT[0:EMB, :],
                    rhs=ktT[0:EMB, :],
                    start=True,
                    stop=True,
                ).then_inc(ts, 1)  # ts 8
                t.matmul(
                    logR1_ps,
                    lhsT=qT[EMB:128, :],
                    rhs=ktT[EMB:128, :],
                    start=True,
                    stop=True,
                ).then_inc(ts, 1)  # ts 9
                t.wait_ge(vs, 7)
                t.matmul(log0_ps, lhsT=prod[:], rhs=hmask, start=True, stop=True).then_inc(
                    ts, 1
                )  # ts 10
                t.wait_ge(vs, 10)
                t.matmul(
                    atT0_ps, lhsT=attn[:, 0, 1:], rhs=ident, is_transpose=True
                ).then_inc(ts, 1)  # ts 11
                t.wait_ge(vs, 11)
                t.matmul(
                    atT1_ps, lhsT=attn[:, 1, 1:], rhs=ident, is_transpose=True
                ).then_inc(ts, 1)  # ts 12
                t.wait_ge(asem, 8)
                t.wait_ge(vs, 6)
                t.matmul(
                    ctxR0_ps,
                    lhsT=vl[:, 0:EMB],
                    rhs=atTs[0][:],
                    start=True,
                    stop=True,
                ).then_inc(ts, 1)  # ts 13
                t.wait_ge(asem, 9)
                t.matmul(
                    ctxR1_ps,
                    lhsT=vl[:, EMB:128],
                    rhs=atTs[1][:],
                    start=True,
                    stop=True,
                ).then_inc(ts, 1)  # ts 14
                t.wait_ge(vs, 13)
                t.matmul(
                    ctx0T_ps, lhsT=ctx0bm_full[:], rhs=ident, is_transpose=True
                ).then_inc(ts, 1)  # ts 15
                t.wait_ge(vs, 15)
                t.matmul(oT_ps, lhsT=WoP[0], rhs=ctxT[:, 0, :], start=True, stop=False)
                t.matmul(
                    oT_ps, lhsT=WoP[1], rhs=ctxT[:, 1, :], start=False, stop=True
                ).then_inc(ts, 1)  # ts 16
                t.wait_ge(asem, 11)
                t.matmul(z_ps, lhsT=oTa[:], rhs=WxA, start=True, stop=True).then_inc(
                    ts, 1
                )  # ts 17

            @block.vector
            def _(v):
                v.memset(aug0T[EMB : EMB + 1, :], 1.0)
                v.memset(augR[EMB : EMB + 1, :], 1.0)
                v.memset(oTa[EMB : EMB + 1, :], 1.0).then_inc(vs, 1)  # vs 1
                v.wait_ge(asem, 3)
                v.wait_ge(d64, 16)
                v.tensor_scalar_add(augR[0:EMB, :], PET1, m_vec[:]).then_inc(
                    vs, 1
                )  # vs 2
                v.wait_ge(ts, 4)
                v.tensor_copy(k0T[:], k0_ps).then_inc(vs, 1)  # vs 3
                v.wait_ge(ts, 5)
                v.tensor_copy(v0bm[:], v0_ps).then_inc(vs, 1)  # vs 4
                v.wait_ge(ts, 6)
                v.tensor_copy(ktT[:], kt_ps).then_inc(vs, 1)  # vs 5
                v.wait_ge(ts, 7)
                v.tensor_copy(vl[:], vl_ps).then_inc(vs, 1)  # vs 6
                v.wait_ge(asem, 4)
                v.wait_ge(vs, 3)
                v.tensor_mul(prod[:], qT[:], k0T[:]).then_inc(vs, 1)  # vs 7
                v.wait_ge(asem, 7)
                v.tensor_add(ssum[:], sR[:], e[:, :, 0]).then_inc(vs, 1)  # vs 8
                v.wait_ge(vs, 8)
                v.reciprocal(rsum[:], ssum[:]).then_inc(vs, 1)  # vs 9
                v.wait_ge(vs, 9)
                v.tensor_scalar_mul(attn[:, 0, :], e[:, 0, :], rsum[:, 0:1]).then_inc(
                    vs, 1
                )  # vs 10
                v.tensor_scalar_mul(attn[:, 1, :], e[:, 1, :], rsum[:, 1:2]).then_inc(
                    vs, 1
                )  # vs 11
                v.tensor_scalar(
                    ctx0bm_full[:, 0:EMB], v0bm[:, 0:EMB], e[:, 0, 0:1],
                    rsum[:, 0:1], op0=ALU.mult, op1=ALU.mult,
                ).then_inc(vs, 1)  # vs 12
                v.tensor_scalar(
                    ctx0bm_full[:, EMB:128], v0bm[:, EMB:128], e[:, 1, 0:1],
                    rsum[:, 1:2], op0=ALU.mult, op1=ALU.mult,
                ).then_inc(vs, 1)  # vs 13
                v.wait_ge(ts, 13)
                v.wait_ge(asem, 10)
                v.tensor_add(
                    ctxT[:, 0, :], ctxR0_ps, ctx0T_sb[0:EMB, :]
                ).then_inc(vs, 1)  # vs 14
                v.wait_ge(ts, 14)
                v.tensor_add(
                    ctxT[:, 1, :], ctxR1_ps, ctx0T_sb[EMB:128, :]
                ).then_inc(vs, 1)  # vs 15
                v.wait_ge(asem, 13)
                v.scalar_tensor_tensor(
                    c2[:], t_i[:], 1.0, t_g[:], op0=ALU.add, op1=ALU.mult
                ).then_inc(vs, 1)  # vs 16
                v.wait_ge(asem, 15)
                v.scalar_tensor_tensor(
                    out2[:], t_o[:], 1.0, tanh_c[:], op0=ALU.add, op1=ALU.mult
                ).then_inc(vs, 1)  # vs 17
                v.wait_ge(vs, 17)
                v.tensor_scalar_mul(out_sb[:], out2[:], 0.5).then_inc(vs, 1)  # vs 18

        nc.compile()
    return nc


def _get_nc():
    if "nc" not in _CACHE:
        if os.environ.get("BASS_TILE", "") not in ("", "0"):
            _CACHE["nc"] = _build()
        else:
            _CACHE["nc"] = _build_raw()
    return _CACHE["nc"]


# ---------------------------------------------------------------------------
# host-side packing + execution
# ---------------------------------------------------------------------------


def _pack_common(queries, values, Wi, bi, Wm, bm, Wq, bq, Wk, bk, Wv, bv, Wo, bo):
    f = np.float32
    queries = np.asarray(queries, f)
    values = np.asarray(values, f)

    # x_7 = concat(queries[b,q], values[b,7]) for row b*Q+q, transposed+ones row
    x7 = np.concatenate(
        [queries.reshape(BQ, DIM), np.repeat(values[:, V - 1, :], Q, axis=0)], axis=1
    )
    x7aT = np.concatenate([x7.T, np.ones((1, BQ), f)], axis=0)
    WiA = np.concatenate([np.asarray(Wi, f), (np.asarray(bi, f) + PE[0])[None, :]], 0)
    pk33 = np.concatenate([x7aT, WiA], axis=1).astype(BF16)  # (33, 192)

    WqA = np.concatenate(
        [np.asarray(Wq, f).reshape(EMB, 128), np.asarray(bq, f).reshape(1, 128)], 0
    )
    WkA = np.concatenate(
        [np.asarray(Wk, f).reshape(EMB, 128), np.asarray(bk, f).reshape(1, 128)], 0
    )
    WvA = np.concatenate(
        [np.asarray(Wv, f).reshape(EMB, 128), np.asarray(bv, f).reshape(1, 128)], 0
    )
    pk65_head = np.concatenate([WqA, WkA, WvA], axis=1).astype(BF16)  # (65, 384)

    PET1 = PE[1:].T  # (64 d, 64 l)
    WoP = np.asarray(Wo, f).transpose(1, 0, 2).reshape(EMB, 128)  # (64k, h*d)
    pk64 = np.concatenate([PET1, WoP], axis=1).astype(BF16)  # (64, 192)

    Wm = np.asarray(Wm, f)
    hmask = np.zeros((128, RH), f)
    for h in range(RH):
        hmask[h * EMB : (h + 1) * EMB, h] = 1.0
    pk128 = np.concatenate(
        [Wm[0:128, :], Wm[128:256, :], np.eye(128, dtype=f), np.ones((128, 1), f), hmask],
        axis=1,
    ).astype(BF16)  # (128, 259)

    pkb = np.stack(
        [np.asarray(bm, f).reshape(EMB), np.asarray(bo, f).reshape(EMB)], axis=1
    )  # (64, 2) f32

    return pk33, pk65_head, pk64, pk128, np.ascontiguousarray(pkb)


def kernel(
    queries,
    values,
    Wi,
    bi,
    Wm,
    bm,
    Wq,
    bq,
    Wk,
    bk,
    Wv,
    bv,
    Wo,
    bo,
    Wx,
    bl,
):
    global LAST_EXEC_TIME_NS
    from concourse.bass_utils import run_bass_kernel_spmd

    f = np.float32
    pk33, pk65_head, pk64, pk128, pkb = _pack_common(
        queries, values, Wi, bi, Wm, bm, Wq, bq, Wk, bk, Wv, bv, Wo, bo
    )
    Wx = np.asarray(Wx, f)
    bl = np.asarray(bl, f)

    # per-core slice of Wx/bl: zi, zg, zo gate blocks, CPC columns each
    gate_off = [0, 2 * UNITS, 3 * UNITS]  # zi, zg, zo starts in the 4*UNITS axis
    in_maps = []
    for c in range(N_CORES):
        cols = np.concatenate(
            [np.arange(off + c * CPC, off + (c + 1) * CPC) for off in gate_off]
        )
        WxA = np.concatenate([Wx[:, cols], bl[cols][None, :]], axis=0)
        pk65 = np.concatenate([pk65_head, WxA.astype(BF16)], axis=1)  # (65, 768)
        in_maps.append(
            {
                "pk33": np.ascontiguousarray(pk33),
                "pk65": np.ascontiguousarray(pk65),
                "pk64": np.ascontiguousarray(pk64),
                "pk128": np.ascontiguousarray(pk128),
                "pkb": pkb,
            }
        )

    nc = _get_nc()
    trace = os.environ.get("BASS_TRACE", "") not in ("", "0")
    core_ids = list(range(N_CORES))
    if trace:
        import tempfile

        tmpdir = tempfile.mkdtemp(prefix="bass_trace_")
        _CACHE["trace_dir"] = tmpdir
        try:
            res = run_bass_kernel_spmd(
                nc, in_maps, core_ids=core_ids, trace=True, tmpdir=tmpdir
            )
        except Exception as e:  # profiling infra missing: fall back untraced
            print(f"trace failed ({e!r}); rerunning without trace")
            os.environ["BASS_TRACE"] = "0"
            res = run_bass_kernel_spmd(nc, in_maps, core_ids=core_ids, trace=False)
    else:
        res = run_bass_kernel_spmd(nc, in_maps, core_ids=core_ids, trace=False)
    LAST_EXEC_TIME_NS = res.exec_time_ns

    out_full = np.concatenate([res.results[c]["out"] for c in range(N_CORES)], axis=1)
    return out_full.reshape(-1, Q, DIM).astype(f)
